# revision 38
# baseline (speedup 1.0000x reference)
"""Trainium2 Bass kernel for nn_MultiHeadSelfAttention_55654186222044.

Reference math (per batch b, per "slice" h of the reshaped activations):
    xs  = x[b,:,h*64:(h+1)*64]                  (T=1024, D=64)
    q_i = xs @ Wq[i].T + bq[i]   (per param set i=0..15), same k_i, v_i
    scores_i = q_i.T @ k_i / 8   (64x64, contraction over T!)
    w_i = softmax(scores_i, axis=-1)
    o_i = v_i @ w_i.T ;  cat = concat_i o_i     (T, 1024)
    out[b,h] = cat @ Wf.T + bf                  (T, 1024)

Because attention is over the feature dim, everything collapses through a
65x65 Gram matrix G = xa.T @ xa (xa = [xs, 1]):
    P         = G @ W~k_all                       (65, 1024)
    scT chunk = P_chunk.T @ W~q chunk  -> diagonal 64x64 blocks are
                scores_i^T (softmax axis lands on the psum partition dim)
    M~_i      = exp(scT_i).T @ [Wv_aug_i | bv | 1] (last col = denominator)
    M_i       = M~_i * (1/denom) per row
    N         = M.T @ Wf.T + u64 x bf             (65, 1024)
    out[b,h]  = xa @ N
This cuts FLOPs ~10x vs the naive dataflow. |scores| < ~50 so exp needs no
max-subtraction (f32 psum, bf16 storage).

Sharding: 32 independent (b, h) slices; 8 cores x 4 slices. Core c takes
b = c//4 and heads 4*(c%4)..4*(c%4)+3 so its x columns are contiguous.
Weights replicated, no collectives. Emission is software-pipelined: the
small-matmul stages (G/P/scores/M) of the next slice pair are striped
between the big N/out matmuls of the current pair so the tensor engine
keeps a high streaming duty cycle and its HAM clock gate stays at 2.4 GHz.
"""

import numpy as np
import ml_dtypes

B, T, E, H = 2, 1024, 1024, 16
D = E // H
SCALE = float(np.sqrt(D))
NCORES = 8

_CACHE = {}


def _build_nc():
    from contextlib import ExitStack

    import concourse.bass as bass
    import concourse.mybir as mybir
    import concourse.tile as tile
    from concourse import bacc

    dt = mybir.dt
    AF = mybir.ActivationFunctionType

    nc = bacc.Bacc(None)
    xh_d = nc.declare_dram_parameter("xh", [128, 8, 4, 65], dt.float16, False)
    xt_d = nc.declare_dram_parameter("xt", [65, 4, 1024], dt.float16, False)
    wqt_d = nc.declare_dram_parameter("wqt", [65, 1024], dt.float16, False)
    wkt_d = nc.declare_dram_parameter("wkt", [65, 1024], dt.float16, False)
    wva_d = nc.declare_dram_parameter("wva", [128, 16, 66], dt.bfloat16, False)
    wft_d = nc.declare_dram_parameter("wft", [128, 8, 1024], dt.float16, False)
    bfh_d = nc.declare_dram_parameter("bfh", [1, 1024], dt.float16, False)
    ub_d = nc.declare_dram_parameter("ub", [1, 65], dt.float16, False)
    out_d = nc.declare_dram_parameter("out", [4, 1024, 1024], dt.float32, True)

    with ExitStack() as ctx:
        tc = ctx.enter_context(tile.TileContext(nc))
        consts = ctx.enter_context(tc.tile_pool(name="consts", bufs=1))
        sbp = ctx.enter_context(tc.tile_pool(name="sbp", bufs=2))
        outp = ctx.enter_context(tc.tile_pool(name="outp", bufs=8))
        ps_w = ctx.enter_context(tc.tile_pool(name="ps_w", bufs=2, space="PSUM"))
        ps_o = ctx.enter_context(tc.tile_pool(name="ps_o", bufs=4, space="PSUM"))
        ps_sc = ctx.enter_context(tc.tile_pool(name="ps_sc", bufs=1, space="PSUM"))
        ps_sm = ctx.enter_context(tc.tile_pool(name="ps_sm", bufs=1, space="PSUM"))

        # const DMAs ordered by first use: G needs xh only; xt not until out(0)
        xh = consts.tile([128, 8, 4, 65], dt.float16, name="xh")
        nc.sync.dma_start(out=xh[:, 0:4], in_=xh_d[:, 0:4, :, :])
        nc.sync.dma_start(out=xh[:, 4:8], in_=xh_d[:, 4:8, :, :])
        wkt = consts.tile([65, 1024], dt.float16, name="wkt")
        nc.sync.dma_start(out=wkt[:], in_=wkt_d[:, :])
        wqt = consts.tile([65, 1024], dt.float16, name="wqt")
        nc.sync.dma_start(out=wqt[:], in_=wqt_d[:, :])
        wva = consts.tile([128, 16, 66], dt.bfloat16, name="wva")
        nc.sync.dma_start(out=wva[:], in_=wva_d[:, :, :])
        wft = consts.tile([128, 8, 1024], dt.float16, name="wft")
        nc.sync.dma_start(out=wft[:], in_=wft_d[:, :, :])
        bfh = consts.tile([1, 1024], dt.float16, name="bfh")
        nc.sync.dma_start(out=bfh[:], in_=bfh_d[:, :])
        ub = consts.tile([1, 65], dt.float16, name="ub")
        nc.sync.dma_start(out=ub[:], in_=ub_d[:, :])
        xt = consts.tile([65, 4, 1024], dt.float16, name="xt")
        nc.sync.dma_start(out=xt[:], in_=xt_d[:, :, :])

        # PE warmup: dense dummy matmuls run while the input DMAs land, so
        # the HAM clock gate is already at 8/8 when real work starts.
        warm = consts.tile([128, 512], dt.float16, name="warm")
        nc.vector.memset(warm[:], 0.0)
        wps = ps_sc.tile([128, 512], dt.float32, name="warmps", tag="pssc")
        for _ in range(10):
            nc.tensor.matmul(wps[:], warm[:, 0:128], warm[:], start=True, stop=True)

        gsb = {}
        psb = {}
        expC = {}
        rec = {}
        msb = {}
        nsb = {}

        def emit_head(*js):
            """G, P, scoresT+exp, M stages for the given slices.
            Yields between work items (PE-instruction groups)."""
            for j in js:
                gps = ps_sm.tile([65, 65], dt.float32, name=f"gps_{j}", tag="pssm")
                for c in range(8):
                    nc.tensor.matmul(
                        gps[:], xh[:, c, j, :], xh[:, c, j, :],
                        start=(c == 0), stop=(c == 7),
                    )
                    if c == 3:
                        yield
                gsb[j] = sbp.tile([65, 65], dt.float16, name=f"gsb_{j}", tag="gsb")
                nc.vector.tensor_copy(out=gsb[j][:], in_=gps[:])
                yield
            for j in js:
                psb[j] = sbp.tile([65, 1024], dt.float16, name=f"psb_{j}", tag="psb")
                for nh in range(2):
                    pps = ps_w.tile([65, 512], dt.float32, name=f"pps_{j}_{nh}", tag="psw")
                    nc.tensor.matmul(
                        pps[:], gsb[j][:], wkt[:, nh * 512 : (nh + 1) * 512],
                        start=True, stop=True,
                    )
                    if nh == 0:
                        nc.vector.tensor_copy(out=psb[j][:, 0:512], in_=pps[:])
                    else:
                        nc.scalar.copy(out=psb[j][:, 512:1024], in_=pps[:])
                    yield
            for j in js:
                # scT chunks: diag 64x64 blocks of P_chunk.T @ W~q_chunk
                expC[j] = sbp.tile([128, 8, 128], dt.bfloat16, name=f"expC_{j}", tag="expC")
                for t in range(2):
                    scp = ps_sc.tile([128, 512], dt.float32, name=f"scp_{j}_{t}", tag="pssc")
                    for u in range(4):
                        c = 4 * t + u
                        nc.tensor.matmul(
                            scp[:, u * 128 : (u + 1) * 128],
                            psb[j][:, c * 128 : (c + 1) * 128],
                            wqt[:, c * 128 : (c + 1) * 128],
                            start=True, stop=True,
                        )
                        if u == 1:
                            yield
                    nc.scalar.activation(
                        out=expC[j][:, 4 * t : 4 * t + 4, :], in_=scp[:], func=AF.Exp
                    )
                    yield
            for j in js:
                rec[j] = sbp.tile([128, 8], dt.float32, name=f"rec_{j}", tag="rec")
                msb[j] = sbp.tile([128, 8, 65], dt.float16, name=f"msb_{j}", tag="msb")
                for c in range(8):
                    mps = ps_sm.tile([128, 66], dt.float32, name=f"mps_{j}_{c}", tag="pssm")
                    nc.tensor.matmul(
                        mps[0:64, :], expC[j][0:64, c, 0:64], wva[0:64, 2 * c, :],
                        start=True, stop=True,
                    )
                    nc.tensor.matmul(
                        mps[64:128, :], expC[j][64:128, c, 64:128], wva[64:128, 2 * c + 1, :],
                        start=True, stop=True,
                    )
                    nc.vector.reciprocal(out=rec[j][:, c : c + 1], in_=mps[:, 65:66])
                    nc.vector.tensor_scalar_mul(
                        out=msb[j][:, c, :], in0=mps[:, 0:65], scalar1=rec[j][:, c : c + 1]
                    )
                    yield

        def emit_tail(*js):
            """N and out stages for the given slices."""
            for j in js:
                nsb[j] = sbp.tile([65, 1024], dt.float16, name=f"nsb_{j}", tag="nsb")
                for nh in range(2):
                    nsp = ps_w.tile([65, 512], dt.float32, name=f"nsp_{j}_{nh}", tag="psw")
                    for c in range(8):
                        nc.tensor.matmul(
                            nsp[:], msb[j][:, c, :], wft[:, c, nh * 512 : (nh + 1) * 512],
                            start=(c == 0), stop=False,
                        )
                        if c % 2 == 1:
                            yield
                    nc.tensor.matmul(
                        nsp[:], ub[:], bfh[:, nh * 512 : (nh + 1) * 512],
                        start=False, stop=True,
                    )
                    if nh == 0:
                        nc.vector.tensor_copy(out=nsb[j][:, 0:512], in_=nsp[:])
                    else:
                        nc.scalar.copy(out=nsb[j][:, 512:1024], in_=nsp[:])
                    yield
            for j in js:
                for c in range(8):
                    osb = outp.tile([128, 1024], dt.float32, name=f"osb_{j}_{c}", tag="osb")
                    for nh in range(2):
                        ops = ps_o.tile([128, 512], dt.float32, name=f"ops_{j}_{c}_{nh}", tag="pso")
                        nc.tensor.matmul(
                            ops[:], xt[:, j, c * 128 : (c + 1) * 128],
                            nsb[j][:, nh * 512 : (nh + 1) * 512],
                            start=True, stop=True,
                        )
                        if (c + nh) % 2 == 0:
                            nc.vector.tensor_copy(
                                out=osb[:, nh * 512 : (nh + 1) * 512], in_=ops[:]
                            )
                        else:
                            nc.scalar.copy(
                                out=osb[:, nh * 512 : (nh + 1) * 512], in_=ops[:]
                            )
                        yield
                    nc.sync.dma_start(out=out_d[j, c * 128 : (c + 1) * 128, :], in_=osb[:])

        def drain(gen):
            for _ in gen:
                pass

        def stripe(a, b):
            a_live, b_live = True, True
            while a_live or b_live:
                if a_live:
                    a_live = next(a, _SENT) is not _SENT
                if b_live:
                    b_live = next(b, _SENT) is not _SENT

        # software pipeline at slice granularity: every tail (big, copy-heavy)
        # is striped with the next slice's head (small matmuls) so the PE
        # always has dense work and the copy engines drain in parallel.
        drain(emit_head(0))
        for s in range(4):
            if s < 3:
                stripe(emit_tail(s), emit_head(s + 1))
            else:
                drain(emit_tail(s))

    nc.finalize()
    return nc


_SENT = object()


def _prep_weights(Wq, bq, Wk, bk, Wv, bv, Wf, bf):
    wqt = np.zeros((65, 1024), np.float16)
    wqt[:64] = (np.transpose(Wq, (2, 0, 1)).reshape(64, H * D) / SCALE).astype(np.float16)
    wqt[64] = (bq.reshape(H * D) / SCALE).astype(np.float16)
    wkt = np.zeros((65, 1024), np.float16)
    wkt[:64] = np.transpose(Wk, (2, 0, 1)).reshape(64, H * D).astype(np.float16)
    wkt[64] = bk.reshape(H * D).astype(np.float16)
    wva_h = np.zeros((64, 16, 66), ml_dtypes.bfloat16)
    wva_h[:, :, :64] = np.transpose(Wv, (1, 0, 2)).astype(ml_dtypes.bfloat16)
    wva_h[:, :, 64] = bv.T.astype(ml_dtypes.bfloat16)
    wva_h[:, :, 65] = 1.0
    wva = np.concatenate([wva_h, wva_h], axis=0)  # duplicated for row-base-64 matmuls
    wft = np.ascontiguousarray(
        Wf.T.reshape(8, 128, 1024).transpose(1, 0, 2)
    ).astype(np.float16)
    bfh = bf.reshape(1, 1024).astype(np.float16)
    ub = np.zeros((1, 65), np.float16)
    ub[0, 64] = 1.0
    return wqt, wkt, wva, wft, bfh, ub


def _prep_x(xs):
    """xs (1024, 256) f32 -> xh (128, 8, 4, 65) fp16 with ones col,
    xt (65, 4, 1024) fp16 with ones row."""
    x16 = xs.astype(np.float16)
    xh = np.ones((128, 8, 4, 65), np.float16)
    xh[:, :, :, :64] = x16.reshape(8, 128, 4, 64).transpose(1, 0, 2, 3)
    xt = np.ones((65, 4, 1024), np.float16)
    xt[:64] = x16.reshape(1024, 4, 64).transpose(2, 1, 0)
    return xh, xt


def _run(inputs, trace=False, tmpdir=None):
    from concourse.bass_utils import run_bass_kernel_spmd

    if "nc" not in _CACHE:
        _CACHE["nc"] = _build_nc()
    nc = _CACHE["nc"]

    x = np.ascontiguousarray(np.asarray(inputs["x"]), dtype=np.float32)
    wqt, wkt, wva, wft, bfh, ub = _prep_weights(
        *(np.asarray(inputs[k], dtype=np.float32) for k in
          ("Wq", "bq", "Wk", "bk", "Wv", "bv", "Wf", "bf"))
    )
    common = dict(wqt=wqt, wkt=wkt, wva=wva, wft=wft, bfh=bfh, ub=ub)
    in_maps = []
    for c in range(NCORES):
        xs = np.ascontiguousarray(x[c // 4][:, (c % 4) * 256 : (c % 4 + 1) * 256])
        xhc, xtc = _prep_x(xs)
        in_maps.append(dict(xh=xhc, xt=xtc, **common))

    res = run_bass_kernel_spmd(
        nc, in_maps, list(range(NCORES)), trace=trace, tmpdir=tmpdir
    )
    out = np.empty((B, H, T, E), np.float32)
    for c in range(NCORES):
        out[c // 4, 4 * (c % 4) : 4 * (c % 4) + 4] = res.results[c]["out"]
    return out, res.exec_time_ns


def kernel(**inputs) -> np.ndarray:
    out, _ = _run(inputs, trace=False)
    return out


# revision 39
# speedup vs baseline: 1.2591x; 1.2591x over previous
"""Trainium2 Bass kernel for nn_MultiHeadSelfAttention_55654186222044.

Reference math (per batch b, per "slice" h of the reshaped activations):
    xs  = x[b,:,h*64:(h+1)*64]                  (T=1024, D=64)
    q_i = xs @ Wq[i].T + bq[i]   (per param set i=0..15), same k_i, v_i
    scores_i = q_i.T @ k_i / 8   (64x64, contraction over T!)
    w_i = softmax(scores_i, axis=-1)
    o_i = v_i @ w_i.T ;  cat = concat_i o_i     (T, 1024)
    out[b,h] = cat @ Wf.T + bf                  (T, 1024)

Because attention is over the feature dim, everything collapses through a
65x65 Gram matrix G = xa.T @ xa (xa = [xs, 1]):
    P         = G @ W~k_all                       (65, 1024)
    scT chunk = P_chunk.T @ W~q chunk  -> diagonal 64x64 blocks are
                scores_i^T (softmax axis lands on the psum partition dim)
    M~_i      = exp(scT_i).T @ [Wv_aug_i | bv | 1] (last col = denominator)
    M_i       = M~_i * (1/denom) per row
    N         = M.T @ Wf.T + u64 x bf             (65, 1024)
    out[b,h]  = xa @ N
This cuts FLOPs ~10x vs the naive dataflow. |scores| < ~50 so exp needs no
max-subtraction (f32 psum, bf16 storage).

Sharding: 32 independent (b, h) slices; 8 cores x 4 slices. Core c takes
b = c//4 and heads 4*(c%4)..4*(c%4)+3 so its x columns are contiguous.
Weights replicated, no collectives. Emission is software-pipelined: the
small-matmul stages (G/P/scores/M) of the next slice pair are striped
between the big N/out matmuls of the current pair so the tensor engine
keeps a high streaming duty cycle and its HAM clock gate stays at 2.4 GHz.
"""

import numpy as np
import ml_dtypes

B, T, E, H = 2, 1024, 1024, 16
D = E // H
SCALE = float(np.sqrt(D))
NCORES = 8

_CACHE = {}


def _build_nc():
    from contextlib import ExitStack

    import concourse.bass as bass
    import concourse.mybir as mybir
    import concourse.tile as tile
    from concourse import bacc

    dt = mybir.dt
    AF = mybir.ActivationFunctionType

    nc = bacc.Bacc(None)
    xh_d = nc.declare_dram_parameter("xh", [128, 8, 4, 65], dt.float16, False)
    xt_d = nc.declare_dram_parameter("xt", [65, 4, 1024], dt.float16, False)
    wqt_d = nc.declare_dram_parameter("wqt", [65, 1024], dt.float16, False)
    wkt_d = nc.declare_dram_parameter("wkt", [65, 1024], dt.float16, False)
    wva_d = nc.declare_dram_parameter("wva", [128, 16, 66], dt.bfloat16, False)
    wft_d = nc.declare_dram_parameter("wft", [128, 8, 1024], dt.float16, False)
    bfh_d = nc.declare_dram_parameter("bfh", [1, 1024], dt.float16, False)
    ub_d = nc.declare_dram_parameter("ub", [1, 65], dt.float16, False)
    out_d = nc.declare_dram_parameter("out", [4, 1024, 1024], dt.float32, True)

    with ExitStack() as ctx:
        tc = ctx.enter_context(tile.TileContext(nc))
        consts = ctx.enter_context(tc.tile_pool(name="consts", bufs=1))
        sbp = ctx.enter_context(tc.tile_pool(name="sbp", bufs=2))
        outp = ctx.enter_context(tc.tile_pool(name="outp", bufs=8))
        ps_w = ctx.enter_context(tc.tile_pool(name="ps_w", bufs=2, space="PSUM"))
        ps_o = ctx.enter_context(tc.tile_pool(name="ps_o", bufs=3, space="PSUM"))
        ps_sc = ctx.enter_context(tc.tile_pool(name="ps_sc", bufs=1, space="PSUM"))
        ps_sm = ctx.enter_context(tc.tile_pool(name="ps_sm", bufs=2, space="PSUM"))

        # const DMAs ordered by first use: G needs xh only; xt not until out(0)
        xh = consts.tile([128, 8, 4, 65], dt.float16, name="xh")
        nc.sync.dma_start(out=xh[:, 0:4], in_=xh_d[:, 0:4, :, :])
        nc.sync.dma_start(out=xh[:, 4:8], in_=xh_d[:, 4:8, :, :])
        wkt = consts.tile([65, 1024], dt.float16, name="wkt")
        nc.sync.dma_start(out=wkt[:], in_=wkt_d[:, :])
        wqt = consts.tile([65, 1024], dt.float16, name="wqt")
        nc.sync.dma_start(out=wqt[:], in_=wqt_d[:, :])
        wva = consts.tile([128, 16, 66], dt.bfloat16, name="wva")
        nc.sync.dma_start(out=wva[:], in_=wva_d[:, :, :])
        wft = consts.tile([128, 8, 1024], dt.float16, name="wft")
        nc.sync.dma_start(out=wft[:], in_=wft_d[:, :, :])
        bfh = consts.tile([1, 1024], dt.float16, name="bfh")
        nc.sync.dma_start(out=bfh[:], in_=bfh_d[:, :])
        ub = consts.tile([1, 65], dt.float16, name="ub")
        nc.sync.dma_start(out=ub[:], in_=ub_d[:, :])
        xt = consts.tile([65, 4, 1024], dt.float16, name="xt")
        nc.sync.dma_start(out=xt[:], in_=xt_d[:, :, :])

        # PE warmup: dense dummy matmuls run while the input DMAs land, so
        # the HAM clock gate is already at 8/8 when real work starts.
        warm = consts.tile([128, 512], dt.float16, name="warm")
        nc.vector.memset(warm[:], 0.0)
        wps = ps_sc.tile([128, 512], dt.float32, name="warmps", tag="pssc")
        for _ in range(10):
            nc.tensor.matmul(wps[:], warm[:, 0:128], warm[:], start=True, stop=True)

        gsb = {}
        psb = {}
        expC = {}
        rec = {}
        msb = {}
        nsb = {}

        def emit_head(*js):
            """G, P, scoresT+exp, M stages for the given slices.
            Yields between work items (PE-instruction groups)."""
            for j in js:
                gps = ps_sm.tile([65, 65], dt.float32, name=f"gps_{j}", tag="pssm")
                for c in range(8):
                    nc.tensor.matmul(
                        gps[:], xh[:, c, j, :], xh[:, c, j, :],
                        start=(c == 0), stop=(c == 7),
                    )
                    if c == 3:
                        yield
                gsb[j] = sbp.tile([65, 65], dt.float16, name=f"gsb_{j}", tag="gsb")
                nc.vector.tensor_copy(out=gsb[j][:], in_=gps[:])
                yield
            for j in js:
                psb[j] = sbp.tile([65, 1024], dt.float16, name=f"psb_{j}", tag="psb")
                for nh in range(2):
                    pps = ps_w.tile([65, 512], dt.float32, name=f"pps_{j}_{nh}", tag="psw")
                    nc.tensor.matmul(
                        pps[:], gsb[j][:], wkt[:, nh * 512 : (nh + 1) * 512],
                        start=True, stop=True,
                    )
                    if nh == 0:
                        nc.vector.tensor_copy(out=psb[j][:, 0:512], in_=pps[:])
                    else:
                        nc.scalar.copy(out=psb[j][:, 512:1024], in_=pps[:])
                    yield
            for j in js:
                # scT chunks: diag 64x64 blocks of P_chunk.T @ W~q_chunk
                expC[j] = sbp.tile([128, 8, 128], dt.bfloat16, name=f"expC_{j}", tag="expC")
                for t in range(2):
                    scp = ps_sc.tile([128, 512], dt.float32, name=f"scp_{j}_{t}", tag="pssc")
                    for u in range(4):
                        c = 4 * t + u
                        nc.tensor.matmul(
                            scp[:, u * 128 : (u + 1) * 128],
                            psb[j][:, c * 128 : (c + 1) * 128],
                            wqt[:, c * 128 : (c + 1) * 128],
                            start=True, stop=True,
                        )
                        if u == 1:
                            yield
                    nc.scalar.activation(
                        out=expC[j][:, 4 * t : 4 * t + 4, :], in_=scp[:], func=AF.Exp
                    )
                    yield
            for j in js:
                rec[j] = sbp.tile([128, 8], dt.float32, name=f"rec_{j}", tag="rec")
                msb[j] = sbp.tile([128, 8, 65], dt.float16, name=f"msb_{j}", tag="msb")
                for c in range(8):
                    mps = ps_sm.tile([128, 66], dt.float32, name=f"mps_{j}_{c}", tag="pssm")
                    nc.tensor.matmul(
                        mps[0:64, :], expC[j][0:64, c, 0:64], wva[0:64, 2 * c, :],
                        start=True, stop=True,
                    )
                    nc.tensor.matmul(
                        mps[64:128, :], expC[j][64:128, c, 64:128], wva[64:128, 2 * c + 1, :],
                        start=True, stop=True,
                    )
                    nc.vector.reciprocal(out=rec[j][:, c : c + 1], in_=mps[:, 65:66])
                    nc.vector.tensor_scalar_mul(
                        out=msb[j][:, c, :], in0=mps[:, 0:65], scalar1=rec[j][:, c : c + 1]
                    )
                    yield

        def emit_tail(*js):
            """N and out stages for the given slices."""
            for j in js:
                nsb[j] = sbp.tile([65, 1024], dt.float16, name=f"nsb_{j}", tag="nsb")
                for nh in range(2):
                    nsp = ps_w.tile([65, 512], dt.float32, name=f"nsp_{j}_{nh}", tag="psw")
                    for c in range(8):
                        nc.tensor.matmul(
                            nsp[:], msb[j][:, c, :], wft[:, c, nh * 512 : (nh + 1) * 512],
                            start=(c == 0), stop=False,
                        )
                        if c % 2 == 1:
                            yield
                    nc.tensor.matmul(
                        nsp[:], ub[:], bfh[:, nh * 512 : (nh + 1) * 512],
                        start=False, stop=True,
                    )
                    if nh == 0:
                        nc.vector.tensor_copy(out=nsb[j][:, 0:512], in_=nsp[:])
                    else:
                        nc.scalar.copy(out=nsb[j][:, 512:1024], in_=nsp[:])
                    yield
            for j in js:
                for c in range(8):
                    osb = outp.tile([128, 1024], dt.float32, name=f"osb_{j}_{c}", tag="osb")
                    for nh in range(2):
                        ops = ps_o.tile([128, 512], dt.float32, name=f"ops_{j}_{c}_{nh}", tag="pso")
                        nc.tensor.matmul(
                            ops[:], xt[:, j, c * 128 : (c + 1) * 128],
                            nsb[j][:, nh * 512 : (nh + 1) * 512],
                            start=True, stop=True,
                        )
                        if (c + nh) % 2 == 0:
                            nc.vector.tensor_copy(
                                out=osb[:, nh * 512 : (nh + 1) * 512], in_=ops[:]
                            )
                        else:
                            nc.scalar.copy(
                                out=osb[:, nh * 512 : (nh + 1) * 512], in_=ops[:]
                            )
                        yield
                    nc.sync.dma_start(out=out_d[j, c * 128 : (c + 1) * 128, :], in_=osb[:])

        def drain(gen):
            for _ in gen:
                pass

        def stripe(a, b):
            a_live, b_live = True, True
            while a_live or b_live:
                if a_live:
                    a_live = next(a, _SENT) is not _SENT
                if b_live:
                    b_live = next(b, _SENT) is not _SENT

        # software pipeline at slice granularity: every tail (big, copy-heavy)
        # is striped with the next slice's head (small matmuls) so the PE
        # always has dense work and the copy engines drain in parallel.
        drain(emit_head(0))
        for s in range(4):
            if s < 3:
                stripe(emit_tail(s), emit_head(s + 1))
            else:
                drain(emit_tail(s))

    nc.finalize()
    return nc


_SENT = object()


def _prep_weights(Wq, bq, Wk, bk, Wv, bv, Wf, bf):
    wqt = np.zeros((65, 1024), np.float16)
    wqt[:64] = (np.transpose(Wq, (2, 0, 1)).reshape(64, H * D) / SCALE).astype(np.float16)
    wqt[64] = (bq.reshape(H * D) / SCALE).astype(np.float16)
    wkt = np.zeros((65, 1024), np.float16)
    wkt[:64] = np.transpose(Wk, (2, 0, 1)).reshape(64, H * D).astype(np.float16)
    wkt[64] = bk.reshape(H * D).astype(np.float16)
    wva_h = np.zeros((64, 16, 66), ml_dtypes.bfloat16)
    wva_h[:, :, :64] = np.transpose(Wv, (1, 0, 2)).astype(ml_dtypes.bfloat16)
    wva_h[:, :, 64] = bv.T.astype(ml_dtypes.bfloat16)
    wva_h[:, :, 65] = 1.0
    wva = np.concatenate([wva_h, wva_h], axis=0)  # duplicated for row-base-64 matmuls
    wft = np.ascontiguousarray(
        Wf.T.reshape(8, 128, 1024).transpose(1, 0, 2)
    ).astype(np.float16)
    bfh = bf.reshape(1, 1024).astype(np.float16)
    ub = np.zeros((1, 65), np.float16)
    ub[0, 64] = 1.0
    return wqt, wkt, wva, wft, bfh, ub


def _prep_x(xs):
    """xs (1024, 256) f32 -> xh (128, 8, 4, 65) fp16 with ones col,
    xt (65, 4, 1024) fp16 with ones row."""
    x16 = xs.astype(np.float16)
    xh = np.ones((128, 8, 4, 65), np.float16)
    xh[:, :, :, :64] = x16.reshape(8, 128, 4, 64).transpose(1, 0, 2, 3)
    xt = np.ones((65, 4, 1024), np.float16)
    xt[:64] = x16.reshape(1024, 4, 64).transpose(2, 1, 0)
    return xh, xt


def _run(inputs, trace=False, tmpdir=None):
    from concourse.bass_utils import run_bass_kernel_spmd

    if "nc" not in _CACHE:
        _CACHE["nc"] = _build_nc()
    nc = _CACHE["nc"]

    x = np.ascontiguousarray(np.asarray(inputs["x"]), dtype=np.float32)
    wqt, wkt, wva, wft, bfh, ub = _prep_weights(
        *(np.asarray(inputs[k], dtype=np.float32) for k in
          ("Wq", "bq", "Wk", "bk", "Wv", "bv", "Wf", "bf"))
    )
    common = dict(wqt=wqt, wkt=wkt, wva=wva, wft=wft, bfh=bfh, ub=ub)
    in_maps = []
    for c in range(NCORES):
        xs = np.ascontiguousarray(x[c // 4][:, (c % 4) * 256 : (c % 4 + 1) * 256])
        xhc, xtc = _prep_x(xs)
        in_maps.append(dict(xh=xhc, xt=xtc, **common))

    res = run_bass_kernel_spmd(
        nc, in_maps, list(range(NCORES)), trace=trace, tmpdir=tmpdir
    )
    out = np.empty((B, H, T, E), np.float32)
    for c in range(NCORES):
        out[c // 4, 4 * (c % 4) : 4 * (c % 4) + 4] = res.results[c]["out"]
    return out, res.exec_time_ns


def kernel(**inputs) -> np.ndarray:
    out, _ = _run(inputs, trace=False)
    return out
